# revision 33
# baseline (speedup 1.0000x reference)
"""Causal self-attention on 8 trn2 NeuronCores.

Problem: B=2, T=2048, C=1024, 16 heads of 64. Sharding: core = 4*b + g
(b = batch, g = head-group of 4 heads). Each core computes QKV projection
for its 4 heads, causal attention, and a partial c_proj (its 256 rows of
w_proj). Host sums the 4 partials per batch (the "all-reduce") + b_proj.

v2 design (vs the fp32r phase-separated baseline):
  * all matmul operands bf16 (same 1 cycle/row PE rate as fp32r, but half
    the DMA bytes and SBUF); PSUM accumulation stays fp32.
  * one continuous PE instruction stream: the attention S->exp->AV chain
    is software-pipelined (AV trails exp by one tk tile) and the
    independent projection work (V tiles, q23/k23 chunks, c_proj tiles)
    is interleaved as "filler" so the PE never waits on the ScalarE exp
    (ScalarE ~73us total vs ~116us of PE work).
  * S matmuls trimmed to the causal region (rhs starts at the diagonal).
  * diagonal-block masking via a PE matmul that accumulates a
    precomputed [128,128] 0/-30000 bias into the S PSUM group through an
    identity lhsT (exp then underflows to exactly 0) - no gpsimd in the
    dependency chain.
  * softmax denominator fused into AV via a ones-column on V ([65,512]
    accumulators); normalization: DVE reciprocal + Pool partition
    broadcast + DVE multiply.

Per-core layouts (partition dim first):
  xT      (1024, 2048) bf16  x[b]^T; 8 chunks (128, 2048)
  q^T/k^T (256, 2048) bf16   qk[m]: m=0,1 q pairs; 2,3 k pairs.
                             1/sqrt(hs) folded into Wq,bq.
  V'      (2048, 16, 4, 65) bf16  natural + ones column
  S^T     (128, 2, 512) PSUM per tk tile (h2-major halves of one chunk)
  y^T     (256, 2048) bf16   normalized attention out
  out     (2048, 1024) f32   partial x@..@w_proj; host adds partials+bias
"""

import numpy as np

import concourse.tile as tile
from concourse import bacc, mybir
from concourse.bass_utils import run_bass_kernel_spmd

B, T, C = 2, 2048, 1024
HS = 64
NCORES = 8
TQC = 512          # tq chunk (attention N granularity)
NT = T // 128      # 16 tk tiles
NG = T // TQC      # 4 tq chunks
F32 = mybir.dt.float32
BF16 = mybir.dt.bfloat16
NPBF16 = mybir.dt.np(mybir.dt.bfloat16)


def build_program():
    nc = bacc.Bacc("TRN2", target_bir_lowering=False, debug=False)

    xT_d = nc.dram_tensor("xT", [C, T], BF16, kind="ExternalInput").ap()
    wqk_d = nc.dram_tensor("wqk", [C, 512], BF16, kind="ExternalInput").ap()
    wv_d = nc.dram_tensor("wv", [C, 256], BF16, kind="ExternalInput").ap()
    wp_d = nc.dram_tensor("wp", [256, 1024], BF16, kind="ExternalInput").ap()
    bqk_d = nc.dram_tensor("bqk", [128, 4], F32, kind="ExternalInput").ap()
    bvb_d = nc.dram_tensor("bvb", [128, 320], F32, kind="ExternalInput").ap()
    im_d = nc.dram_tensor("imask", [128, 256], BF16, kind="ExternalInput").ap()
    out_d = nc.dram_tensor("out", [T, C], BF16, kind="ExternalOutput").ap()

    with tile.TileContext(nc) as tc:
        _kernel(tc, out_d, xT_d, wqk_d, wv_d, wp_d, bqk_d, bvb_d, im_d)
    nc.compile()
    return nc


def _kernel(tc, out_d, xT_d, wqk_d, wv_d, wp_d, bqk_d, bvb_d, im_d):
    nc = tc.nc
    AF = mybir.ActivationFunctionType

    with (
        tc.tile_pool(name="persist", bufs=1) as pers,
        tc.tile_pool(name="ptp", bufs=4) as ptp,
        tc.tile_pool(name="stgp", bufs=4) as stgp,
        tc.tile_pool(name="lrp", bufs=4) as lrp,
        tc.tile_pool(name="lbp", bufs=4) as lbp,
    ):
        # weights/biases on the SWDGE (gpsimd) queue, x on the HWDGE (sync)
        # queue: the rings drain in parallel, and the DMA bus is a serial
        # resource, so order by first use: wqk (gates the first matmul)
        # ahead of everything else; bvb (the largest constant) last.
        wqk3 = wqk_d.rearrange("(c p) m -> c p m", p=128)
        wqk = []
        for c in range(8):
            t_ = pers.tile([128, 512], BF16, tag=f"wqk{c}", name=f"wqk{c}")
            nc.gpsimd.dma_start(out=t_, in_=wqk3[c])
            wqk.append(t_)
        bqk = pers.tile([128, 4], F32, tag="bqk")
        nc.gpsimd.dma_start(out=bqk, in_=bqk_d)
        imask = pers.tile([128, 256], BF16, tag="imask")
        nc.gpsimd.dma_start(out=imask, in_=im_d)
        bvb = pers.tile([128, 320], F32, tag="bvb")
        nc.gpsimd.dma_start(out=bvb, in_=bvb_d)
        wv3 = wv_d.rearrange("(c p) m -> c p m", p=128)
        wv = []
        for c in range(8):
            t_ = pers.tile([128, 256], BF16, tag=f"wv{c}", name=f"wv{c}")
            nc.gpsimd.dma_start(out=t_, in_=wv3[c])
            wv.append(t_)
        wp3 = wp_d.rearrange("(c p) m -> c p m", p=128)
        wp = []
        for c in range(2):
            t_ = pers.tile([128, 1024], BF16, tag=f"wp{c}", name=f"wp{c}")
            nc.gpsimd.dma_start(out=t_, in_=wp3[c])
            wp.append(t_)
        xT3 = xT_d.rearrange("(c p) t -> c p t", p=128)
        xt = []
        for c in range(8):
            t_ = pers.tile([128, T], BF16, tag=f"xt{c}", name=f"xt{c}")
            nc.sync.dma_start(out=t_, in_=xT3[c])
            xt.append(t_)

        # q^T / k^T chunks: m=0,1 -> q pairs, m=2,3 -> k pairs
        qk = [pers.tile([128, T], BF16, tag=f"qk{m}", name=f"qk{m}")
              for m in range(4)]
        # touch ScalarE once so the Identity/Exp activation table loads
        # during the DMA-bound startup instead of before the first bias add
        warm = pers.tile([1, 2], F32, tag="warm")
        nc.vector.memset(warm[:], 0.0)
        nc.scalar.activation(out=warm[0:1, 0:1], in_=warm[0:1, 1:2],
                             func=mybir.ActivationFunctionType.Exp)
        # V' = [V | 1] per (tk-tile, head); ones column from bvb[:, 256:320]
        v_all = pers.tile([128, NT, 4, HS + 1], BF16, tag="v_all",
                          name="v_all")
        nc.vector.tensor_copy(
            out=v_all[:, :, :, HS],
            in_=bvb[:, 256:320].rearrange("p (a b) -> p a b", a=NT),
        )
        # y^T chunks (normalized attention output), pair-stacked
        yt = [pers.tile([128, T], BF16, tag=f"yt{p}", name=f"yt{p}")
              for p in range(2)]

        # ---- P1: q01 (m=0) + k01 (m=2), c-outer so the PE streams right
        # behind the x DMA. 8 PSUM banks, then released for attention.
        p1 = tc.alloc_tile_pool(name="p1", bufs=8, space="PSUM")
        # slot order matches the bias-read order so the v tiles below
        # (which rotate onto slots 0..3) wait on the earliest-freed slots
        pst = {0: [None] * 4, 2: [None] * 4}
        for i in range(4):
            for m in (0, 2):
                pst[m][i] = p1.tile([128, 512], F32, tag="p1",
                                    name=f"p1_{m}_{i}")
        for c in range(8):
            for m in (0, 2):
                lhsT = wqk[c][:, 128 * m:128 * (m + 1)]
                for i in range(4):
                    nc.tensor.matmul(
                        pst[m][i][:],
                        lhsT,
                        xt[c][:, 512 * i:512 * (i + 1)],
                        start=(c == 0),
                        stop=(c == 7),
                    )
        # bias on ScalarE (Identity+bias is ~3x cheaper than the DVE
        # tensor-scalar op and DVE is needed for drains); i-minor order so
        # the first S matmul (needs m0/i0 + m2/i0) unblocks first.
        for i in range(4):
            for m in (0, 2):
                nc.scalar.add(
                    out=qk[m][:, 512 * i:512 * (i + 1)],
                    in_=pst[m][i][:],
                    add=bqk[:, m:m + 1],
                )

        # ---- one V tile: natural layout [tk 128, 4 heads x 64] + bias ----
        def v_unit(pool, t, tag):
            vp = pool.tile([128, 256], F32, tag=tag, name=f"vp{t}")
            for c in range(8):
                nc.tensor.matmul(
                    vp[:],
                    xt[c][:, 128 * t:128 * (t + 1)],
                    wv[c][:],
                    start=(c == 0),
                    stop=(c == 7),
                )
            nc.vector.tensor_add(
                out=v_all[:, t, :, 0:HS],
                in0=vp[:].rearrange("p (h d) -> p h d", h=4),
                in1=bvb[:, 0:256].rearrange("p (h d) -> p h d", h=4),
            )

        # v tiles 0..3 (needed by attention chunk 0) out of the p1 pool
        for t in range(4):
            v_unit(p1, t, "p1")
        p1.release()

        # attention-era PSUM: st 2x[128,2,512] (4 banks) + opr 2x[128,512]
        # (2 banks) + po 2x[128,512] (2 banks) = 8 banks exactly.
        stP = tc.alloc_tile_pool(name="stP", bufs=2, space="PSUM")
        oprP = tc.alloc_tile_pool(name="oprP", bufs=2, space="PSUM")
        poP = tc.alloc_tile_pool(name="poP", bufs=2, space="PSUM")

        # ---- filler units (independent PE work pumped into the attention
        # stream so the PE never stalls on ScalarE exp) ----
        def mqk_unit(m, i):
            ps = poP.tile([128, 512], F32, tag="po", name=f"m{m}_{i}")
            lo = 128 * m
            for c in range(8):
                nc.tensor.matmul(
                    ps[:],
                    wqk[c][:, lo:lo + 128],
                    xt[c][:, 512 * i:512 * (i + 1)],
                    start=(c == 0),
                    stop=(c == 7),
                )
            nc.scalar.add(
                out=qk[m][:, 512 * i:512 * (i + 1)],
                in_=ps[:],
                add=bqk[:, m:m + 1],
            )

        def proj_unit(tt, pool=None, tag="po"):
            if pool is None:
                pool = poP
            if tag == "st":
                big = pool.tile([128, 2, 512], F32, tag="st", name=f"pp{tt}")
                pp = [big[:, oc, :] for oc in range(2)]
            else:
                pp = [pool.tile([128, 512], F32, tag=tag, name=f"pp{tt}_{oc}")
                      for oc in range(2)]
            stg = stgp.tile([128, 1024], BF16, tag="stg", name=f"stg{tt}")
            for oc in range(2):
                for p2 in range(2):
                    nc.tensor.matmul(
                        pp[oc][:],
                        yt[p2][:, 128 * tt:128 * (tt + 1)],
                        wp[p2][:, 512 * oc:512 * (oc + 1)],
                        start=(p2 == 0),
                        stop=(p2 == 1),
                    )
                # NB: GPSIMD/Pool cannot access PSUM (BIR verifier), so the
                # staging copies go on DVE + ScalarE.
                if oc == 0:
                    nc.vector.tensor_copy(out=stg[:, 0:512], in_=pp[0][:])
                else:
                    nc.scalar.copy(out=stg[:, 512:1024], in_=pp[1][:])
            # alternate output queues so tail DMA issues overlap
            eng = nc.scalar if (tail_mode[0] and tt % 2) else nc.sync
            eng.dma_start(out=out_d[128 * tt:128 * (tt + 1), :], in_=stg[:])

        # filler ledger: emit filler PE work only while the attention
        # stream's ScalarE (exp) time exceeds its PE time, so filler lasts
        # the whole window instead of exhausting early. Costs are
        # cost-model estimates (ns) at full PE clock.
        fillers = []
        for t in range(4, NT):
            fillers.append(("v", t, 2048 * 0.42))
        for i in range(2):
            for m in (1, 3):
                fillers.append(("m", (m, i), 4096 * 0.42))
        # i=2,3 deferred past the pair-0 flush: they feed only pair-1
        # chunks 2,3 and keep the pair-1 window supplied with filler
        deferred = []
        m_deadline = {}
        for i in (2, 3):
            for m in (1, 3):
                deferred.append(("m", (m, i), 4096 * 0.42))
            m_deadline[i] = len(deferred) - 1
        fill_pos = [0]
        ledger = [0.0]  # accumulated (exp - attention-PE) deficit, ns
        tail_mode = [False]  # tail proj units overlap 2-deep via the st banks

        def emit_filler(f):
            kind, a, _ = f
            if kind == "v":
                v_unit(poP, a, "po")
            elif kind == "m":
                mqk_unit(*a)
            elif tail_mode[0]:
                # alternate the freed attention pools: 4 tail proj tiles
                # overlap instead of serializing on 2 PSUM banks
                if tail_mode[0] % 2:
                    proj_unit(a, stP, "st")
                else:
                    proj_unit(a, oprP, "opr")
                tail_mode[0] += 1
            else:
                proj_unit(a)

        reserve = [0]  # fillers held back from pump (tail-drain cover)

        def pump():
            while (fill_pos[0] < len(fillers) - reserve[0]
                   and ledger[0] > 0):
                f = fillers[fill_pos[0]]
                emit_filler(f)
                ledger[0] -= f[2]
                fill_pos[0] += 1

        def flush():
            while fill_pos[0] < len(fillers):
                emit_filler(fillers[fill_pos[0]])
                fill_pos[0] += 1

        # ---- attention: per (pair, chunk g): S->exp pipelined one tk tile
        # ahead of AV; drains feed y^T; exp(S+maskbias) handles the
        # causal diagonal (masked entries underflow to exactly 0).
        def attn_chunk(pair, g):
            last = 4 * g + 3
            opr = [oprP.tile([128, 512], F32, tag="opr",
                             name=f"opr{pair}_{g}_{h2}")
                   for h2 in range(2)]
            pend = None  # (t, sub0, pt)

            def emit_av(t, sub0, pt):
                if pair == 0 and t >= 4:
                    # v tile t is filler index t-4; it must precede this AV
                    while fill_pos[0] <= t - 4:
                        emit_filler(fillers[fill_pos[0]])
                        fill_pos[0] += 1
                for h2 in range(2):
                    nc.tensor.matmul(
                        opr[h2][0:HS + 1, sub0:512],
                        v_all[:, t, 2 * pair + h2, :],
                        pt[:, h2, sub0:512],
                        start=(t == 0),
                        stop=(t == last),
                    )

            for t in range(last + 1):
                sub0 = max(0, 128 * t - TQC * g)
                diag = t >= 4 * g
                st = stP.tile([128, 2, 512], F32, tag="st",
                              name=f"st{pair}_{g}_{t}")
                pt = ptp.tile([128, 2, 512], BF16, tag="pt",
                              name=f"pt{pair}_{g}_{t}")
                for h2 in range(2):
                    pb = 64 * h2
                    nc.tensor.matmul(
                        st[:, h2, sub0:512],
                        qk[2 + pair][pb:pb + 64, 128 * t:128 * (t + 1)],
                        qk[pair][pb:pb + 64, TQC * g + sub0:TQC * (g + 1)],
                        start=True,
                        stop=not diag,
                    )
                    if diag:
                        nc.tensor.matmul(
                            st[:, h2, sub0:sub0 + 128],
                            imask[:, 0:128],
                            imask[:, 128:256],
                            start=False,
                            stop=True,
                        )
                nc.scalar.activation(
                    out=pt[:, :, sub0:512], in_=st[:, :, sub0:512],
                    func=AF.Exp,
                )
                w = 512 - sub0
                exp_ns = 2 * w * 0.833 + 185
                pe_ns = 4 * w * 0.42 + (128 * 2 * 0.42 if diag else 0)
                ledger[0] += exp_ns - pe_ns
                if t == 0:
                    ledger[0] += 3200  # cover the previous chunk's drain
                pump()
                if pend is not None:
                    emit_av(*pend)
                pend = (t, sub0, pt)
            emit_av(*pend)

            # normalize: 1/l broadcast over the 64 head rows
            for h2 in range(2):
                lr = lrp.tile([1, 512], F32, tag="lr",
                              name=f"lr{pair}_{g}_{h2}")
                nc.vector.reciprocal(out=lr[:], in_=opr[h2][HS:HS + 1, :])
                lb = lbp.tile([64, 512], F32, tag="lb",
                              name=f"lb{pair}_{g}_{h2}")
                nc.gpsimd.partition_broadcast(lb[:], lr[:], channels=64)
                nc.vector.tensor_mul(
                    out=yt[pair][64 * h2:64 * (h2 + 1),
                                 TQC * g:TQC * (g + 1)],
                    in0=opr[h2][0:HS, :],
                    in1=lb[:],
                )

        for g in range(NG):
            attn_chunk(0, g)
        flush()  # remaining v / m(i<2) fillers
        base = len(fillers)
        fillers.extend(deferred)
        for g in range(NG):
            if g in m_deadline:  # qk[1]/qk[3] chunk i=g must exist now
                while fill_pos[0] <= base + m_deadline[g]:
                    emit_filler(fillers[fill_pos[0]])
                    fill_pos[0] += 1
            if g == NG - 1:
                # hold 2 proj tiles back: they don't depend on the final
                # drain, so they cover its latency in the tail
                reserve[0] = 2
            attn_chunk(1, g)
            for tt in range(4 * g, 4 * g + 4):
                fillers.append(("p", tt, 2048 * 0.42))
        tail_mode[0] = 1
        flush()
        poP.release()
        oprP.release()
        stP.release()


_PROG = None


def _get_program():
    global _PROG
    if _PROG is None:
        _PROG = build_program()
    return _PROG


def make_in_maps(x, w_attn, b_attn, w_proj, b_proj):
    x = np.asarray(x, dtype=np.float32)
    w_attn = np.asarray(w_attn, dtype=np.float32)
    b_attn = np.asarray(b_attn, dtype=np.float32)
    w_proj = np.asarray(w_proj, dtype=np.float32)
    s = 1.0 / np.sqrt(HS)
    wq, wk, wv = w_attn[:, 0:C], w_attn[:, C:2 * C], w_attn[:, 2 * C:3 * C]
    bq, bk, bv = b_attn[0:C], b_attn[C:2 * C], b_attn[2 * C:3 * C]
    tri = np.triu(np.ones((128, 128), dtype=np.float32))  # keep tk <= tq
    imask = np.concatenate(
        [np.eye(128, dtype=np.float32), (tri - 1.0) * 30000.0], axis=1)
    in_maps = []
    for core in range(NCORES):
        b, g = divmod(core, 4)
        cs = slice(256 * g, 256 * (g + 1))
        bqk_ = np.stack([bq[cs][0:128] * s, bq[cs][128:256] * s,
                         bk[cs][0:128], bk[cs][128:256]], axis=1)
        in_maps.append({
            "xT": np.ascontiguousarray(x[b].T).astype(NPBF16),
            "wqk": np.ascontiguousarray(
                np.concatenate([wq[:, cs] * s, wk[:, cs]],
                               axis=1)).astype(NPBF16),
            "wv": np.ascontiguousarray(wv[:, cs]).astype(NPBF16),
            "wp": np.ascontiguousarray(w_proj[cs, :]).astype(NPBF16),
            "bqk": np.ascontiguousarray(bqk_),
            "bvb": np.ascontiguousarray(np.concatenate([
                np.broadcast_to(bv[cs][None, :], (128, 256)),
                np.ones((128, 64), dtype=np.float32)], axis=1)),
            "imask": imask.astype(NPBF16),
        })
    return in_maps


def gather_output(results, b_proj):
    b_proj = np.asarray(b_proj, dtype=np.float32)
    out = np.empty((B, T, C), dtype=np.float32)
    for b in range(B):
        acc = results[4 * b]["out"].astype(np.float32)
        for g in range(1, 4):
            acc = acc + results[4 * b + g]["out"].astype(np.float32)
        out[b] = acc + b_proj[None, :]
    return out


def kernel(x, w_attn, b_attn, w_proj, b_proj):
    nc = _get_program()
    in_maps = make_in_maps(x, w_attn, b_attn, w_proj, b_proj)
    res = run_bass_kernel_spmd(nc, in_maps, core_ids=list(range(NCORES)))
    return gather_output(res.results, b_proj)


# revision 34
# speedup vs baseline: 1.0233x; 1.0233x over previous
"""Causal self-attention on 8 trn2 NeuronCores.

Problem: B=2, T=2048, C=1024, 16 heads of 64. Sharding: core = 4*b + g
(b = batch, g = head-group of 4 heads). Each core computes QKV projection
for its 4 heads, causal attention, and a partial c_proj (its 256 rows of
w_proj). Host sums the 4 partials per batch (the "all-reduce") + b_proj.

v2 design (vs the fp32r phase-separated baseline):
  * all matmul operands bf16 (same 1 cycle/row PE rate as fp32r, but half
    the DMA bytes and SBUF); PSUM accumulation stays fp32.
  * one continuous PE instruction stream: the attention S->exp->AV chain
    is software-pipelined (AV trails exp by one tk tile) and the
    independent projection work (V tiles, q23/k23 chunks, c_proj tiles)
    is interleaved as "filler" so the PE never waits on the ScalarE exp
    (ScalarE ~73us total vs ~116us of PE work).
  * S matmuls trimmed to the causal region (rhs starts at the diagonal).
  * diagonal-block masking via a PE matmul that accumulates a
    precomputed [128,128] 0/-30000 bias into the S PSUM group through an
    identity lhsT (exp then underflows to exactly 0) - no gpsimd in the
    dependency chain.
  * softmax denominator fused into AV via a ones-column on V ([65,512]
    accumulators); normalization: DVE reciprocal + Pool partition
    broadcast + DVE multiply.

Per-core layouts (partition dim first):
  xT      (1024, 2048) bf16  x[b]^T; 8 chunks (128, 2048)
  q^T/k^T (256, 2048) bf16   qk[m]: m=0,1 q pairs; 2,3 k pairs.
                             1/sqrt(hs) folded into Wq,bq.
  V'      (2048, 16, 4, 65) bf16  natural + ones column
  S^T     (128, 2, 512) PSUM per tk tile (h2-major halves of one chunk)
  y^T     (256, 2048) bf16   normalized attention out
  out     (2048, 1024) f32   partial x@..@w_proj; host adds partials+bias
"""

import numpy as np

import concourse.tile as tile
from concourse import bacc, mybir
from concourse.bass_utils import run_bass_kernel_spmd

B, T, C = 2, 2048, 1024
HS = 64
NCORES = 8
TQC = 512          # tq chunk (attention N granularity)
NT = T // 128      # 16 tk tiles
NG = T // TQC      # 4 tq chunks
F32 = mybir.dt.float32
BF16 = mybir.dt.bfloat16
NPBF16 = mybir.dt.np(mybir.dt.bfloat16)


def build_program():
    nc = bacc.Bacc("TRN2", target_bir_lowering=False, debug=False)

    xT_d = nc.dram_tensor("xT", [C, T], BF16, kind="ExternalInput").ap()
    wqk_d = nc.dram_tensor("wqk", [C, 512], BF16, kind="ExternalInput").ap()
    wv_d = nc.dram_tensor("wv", [C, 256], BF16, kind="ExternalInput").ap()
    wp_d = nc.dram_tensor("wp", [256, 1024], BF16, kind="ExternalInput").ap()
    bqk_d = nc.dram_tensor("bqk", [128, 4], F32, kind="ExternalInput").ap()
    bvb_d = nc.dram_tensor("bvb", [128, 320], F32, kind="ExternalInput").ap()
    im_d = nc.dram_tensor("imask", [128, 256], BF16, kind="ExternalInput").ap()
    out_d = nc.dram_tensor("out", [T, C], BF16, kind="ExternalOutput").ap()

    with tile.TileContext(nc) as tc:
        _kernel(tc, out_d, xT_d, wqk_d, wv_d, wp_d, bqk_d, bvb_d, im_d)
    nc.compile()
    return nc


def _kernel(tc, out_d, xT_d, wqk_d, wv_d, wp_d, bqk_d, bvb_d, im_d):
    nc = tc.nc
    AF = mybir.ActivationFunctionType

    with (
        tc.tile_pool(name="persist", bufs=1) as pers,
        tc.tile_pool(name="ptp", bufs=4) as ptp,
        tc.tile_pool(name="stgp", bufs=4) as stgp,
        tc.tile_pool(name="lrp", bufs=4) as lrp,
        tc.tile_pool(name="lbp", bufs=4) as lbp,
    ):
        # weights/biases on the SWDGE (gpsimd) queue, x on the HWDGE (sync)
        # queue: the rings drain in parallel, and the DMA bus is a serial
        # resource, so order by first use: wqk (gates the first matmul)
        # ahead of everything else; bvb (the largest constant) last.
        wqk3 = wqk_d.rearrange("(c p) m -> c p m", p=128)
        wqk = []
        for c in range(8):
            t_ = pers.tile([128, 512], BF16, tag=f"wqk{c}", name=f"wqk{c}")
            nc.gpsimd.dma_start(out=t_, in_=wqk3[c])
            wqk.append(t_)
        bqk = pers.tile([128, 4], F32, tag="bqk")
        nc.gpsimd.dma_start(out=bqk, in_=bqk_d)
        imask = pers.tile([128, 256], BF16, tag="imask")
        nc.gpsimd.dma_start(out=imask, in_=im_d)
        bvb = pers.tile([128, 320], F32, tag="bvb")
        nc.gpsimd.dma_start(out=bvb, in_=bvb_d)
        wv3 = wv_d.rearrange("(c p) m -> c p m", p=128)
        wv = []
        for c in range(8):
            t_ = pers.tile([128, 256], BF16, tag=f"wv{c}", name=f"wv{c}")
            nc.gpsimd.dma_start(out=t_, in_=wv3[c])
            wv.append(t_)
        wp3 = wp_d.rearrange("(c p) m -> c p m", p=128)
        wp = []
        for c in range(2):
            t_ = pers.tile([128, 1024], BF16, tag=f"wp{c}", name=f"wp{c}")
            nc.gpsimd.dma_start(out=t_, in_=wp3[c])
            wp.append(t_)
        xT3 = xT_d.rearrange("(c p) t -> c p t", p=128)
        xt = []
        for c in range(8):
            t_ = pers.tile([128, T], BF16, tag=f"xt{c}", name=f"xt{c}")
            nc.sync.dma_start(out=t_, in_=xT3[c])
            xt.append(t_)

        # q^T / k^T chunks: m=0,1 -> q pairs, m=2,3 -> k pairs
        qk = [pers.tile([128, T], BF16, tag=f"qk{m}", name=f"qk{m}")
              for m in range(4)]
        # touch ScalarE once so the Identity/Exp activation table loads
        # during the DMA-bound startup instead of before the first bias add
        warm = pers.tile([1, 2], F32, tag="warm")
        nc.vector.memset(warm[:], 0.0)
        nc.scalar.activation(out=warm[0:1, 0:1], in_=warm[0:1, 1:2],
                             func=mybir.ActivationFunctionType.Exp)
        # V' = [V | 1] per (tk-tile, head); ones column from bvb[:, 256:320]
        v_all = pers.tile([128, NT, 4, HS + 1], BF16, tag="v_all",
                          name="v_all")
        nc.vector.tensor_copy(
            out=v_all[:, :, :, HS],
            in_=bvb[:, 256:320].rearrange("p (a b) -> p a b", a=NT),
        )
        # y^T chunks (normalized attention output), pair-stacked
        yt = [pers.tile([128, T], BF16, tag=f"yt{p}", name=f"yt{p}")
              for p in range(2)]

        # ---- P1: q01 (m=0) + k01 (m=2), c-outer so the PE streams right
        # behind the x DMA. 8 PSUM banks, then released for attention.
        p1 = tc.alloc_tile_pool(name="p1", bufs=8, space="PSUM")
        # slot order matches the bias-read order so the v tiles below
        # (which rotate onto slots 0..3) wait on the earliest-freed slots
        pst = {0: [None] * 4, 2: [None] * 4}
        for i in range(4):
            for m in (0, 2):
                pst[m][i] = p1.tile([128, 512], F32, tag="p1",
                                    name=f"p1_{m}_{i}")
        for c in range(8):
            for m in (0, 2):
                lhsT = wqk[c][:, 128 * m:128 * (m + 1)]
                for i in range(4):
                    nc.tensor.matmul(
                        pst[m][i][:],
                        lhsT,
                        xt[c][:, 512 * i:512 * (i + 1)],
                        start=(c == 0),
                        stop=(c == 7),
                    )
        # bias on ScalarE (Identity+bias is ~3x cheaper than the DVE
        # tensor-scalar op and DVE is needed for drains); i-minor order so
        # the first S matmul (needs m0/i0 + m2/i0) unblocks first.
        for i in range(4):
            for m in (0, 2):
                nc.scalar.add(
                    out=qk[m][:, 512 * i:512 * (i + 1)],
                    in_=pst[m][i][:],
                    add=bqk[:, m:m + 1],
                )

        # ---- one V tile: natural layout [tk 128, 4 heads x 64] + bias ----
        def v_unit(pool, t, tag):
            vp = pool.tile([128, 256], F32, tag=tag, name=f"vp{t}")
            for c in range(8):
                nc.tensor.matmul(
                    vp[:],
                    xt[c][:, 128 * t:128 * (t + 1)],
                    wv[c][:],
                    start=(c == 0),
                    stop=(c == 7),
                )
            nc.vector.tensor_add(
                out=v_all[:, t, :, 0:HS],
                in0=vp[:].rearrange("p (h d) -> p h d", h=4),
                in1=bvb[:, 0:256].rearrange("p (h d) -> p h d", h=4),
            )

        # v tiles 0..3 (needed by attention chunk 0) out of the p1 pool
        for t in range(4):
            v_unit(p1, t, "p1")
        p1.release()

        # attention-era PSUM: st 2x[128,2,512] (4 banks) + opr 2x[128,512]
        # (2 banks) + po 2x[128,512] (2 banks) = 8 banks exactly.
        stP = tc.alloc_tile_pool(name="stP", bufs=2, space="PSUM")
        oprP = tc.alloc_tile_pool(name="oprP", bufs=2, space="PSUM")
        poP = tc.alloc_tile_pool(name="poP", bufs=2, space="PSUM")

        # ---- filler units (independent PE work pumped into the attention
        # stream so the PE never stalls on ScalarE exp) ----
        def mqk_unit(m, i):
            ps = poP.tile([128, 512], F32, tag="po", name=f"m{m}_{i}")
            lo = 128 * m
            for c in range(8):
                nc.tensor.matmul(
                    ps[:],
                    wqk[c][:, lo:lo + 128],
                    xt[c][:, 512 * i:512 * (i + 1)],
                    start=(c == 0),
                    stop=(c == 7),
                )
            nc.scalar.add(
                out=qk[m][:, 512 * i:512 * (i + 1)],
                in_=ps[:],
                add=bqk[:, m:m + 1],
            )

        def proj_unit(tt, pool=None, tag="po"):
            if pool is None:
                pool = poP
            if tag == "st":
                big = pool.tile([128, 2, 512], F32, tag="st", name=f"pp{tt}")
                pp = [big[:, oc, :] for oc in range(2)]
            else:
                pp = [pool.tile([128, 512], F32, tag=tag, name=f"pp{tt}_{oc}")
                      for oc in range(2)]
            stg = stgp.tile([128, 1024], BF16, tag="stg", name=f"stg{tt}")
            for oc in range(2):
                for p2 in range(2):
                    nc.tensor.matmul(
                        pp[oc][:],
                        yt[p2][:, 128 * tt:128 * (tt + 1)],
                        wp[p2][:, 512 * oc:512 * (oc + 1)],
                        start=(p2 == 0),
                        stop=(p2 == 1),
                    )
                # NB: GPSIMD/Pool cannot access PSUM (BIR verifier), so the
                # staging copies go on DVE + ScalarE.
                if oc == 0:
                    nc.vector.tensor_copy(out=stg[:, 0:512], in_=pp[0][:])
                else:
                    nc.scalar.copy(out=stg[:, 512:1024], in_=pp[1][:])
            # alternate output queues so tail DMA issues overlap
            eng = nc.scalar if (tail_mode[0] and tt % 2) else nc.sync
            eng.dma_start(out=out_d[128 * tt:128 * (tt + 1), :], in_=stg[:])

        # filler ledger: emit filler PE work only while the attention
        # stream's ScalarE (exp) time exceeds its PE time, so filler lasts
        # the whole window instead of exhausting early. Costs are
        # cost-model estimates (ns) at full PE clock.
        fillers = []
        for t in range(4, NT):
            fillers.append(("v", t, 2048 * 0.42))
        for i in range(2):
            for m in (1, 3):
                fillers.append(("m", (m, i), 4096 * 0.42))
        # i=2,3 deferred past the pair-0 flush: they feed only pair-1
        # chunks 2,3 and keep the pair-1 window supplied with filler
        deferred = []
        m_deadline = {}
        for i in (2, 3):
            for m in (1, 3):
                deferred.append(("m", (m, i), 4096 * 0.42))
            m_deadline[i] = len(deferred) - 1
        fill_pos = [0]
        ledger = [0.0]  # accumulated (exp - attention-PE) deficit, ns
        tail_mode = [False]  # tail proj units overlap 2-deep via the st banks

        def emit_filler(f):
            kind, a, _ = f
            if kind == "v":
                v_unit(poP, a, "po")
            elif kind == "m":
                mqk_unit(*a)
            elif tail_mode[0]:
                # alternate the freed attention pools: 4 tail proj tiles
                # overlap instead of serializing on 2 PSUM banks
                if tail_mode[0] % 2:
                    proj_unit(a, stP, "st")
                else:
                    proj_unit(a, oprP, "opr")
                tail_mode[0] += 1
            else:
                proj_unit(a)

        reserve = [0]  # fillers held back from pump (tail-drain cover)

        def pump():
            while (fill_pos[0] < len(fillers) - reserve[0]
                   and ledger[0] > 0):
                f = fillers[fill_pos[0]]
                emit_filler(f)
                ledger[0] -= f[2]
                fill_pos[0] += 1

        def flush():
            while fill_pos[0] < len(fillers):
                emit_filler(fillers[fill_pos[0]])
                fill_pos[0] += 1

        # ---- attention: per (pair, chunk g): S->exp pipelined one tk tile
        # ahead of AV; drains feed y^T; exp(S+maskbias) handles the
        # causal diagonal (masked entries underflow to exactly 0).
        def attn_chunk(pair, g):
            last = 4 * g + 3
            opr = [oprP.tile([128, 512], F32, tag="opr",
                             name=f"opr{pair}_{g}_{h2}")
                   for h2 in range(2)]
            pend = None  # (t, sub0, pt)

            def emit_av(t, sub0, pt):
                if pair == 0 and t >= 4:
                    # v tile t is filler index t-4; it must precede this AV
                    while fill_pos[0] <= t - 4:
                        emit_filler(fillers[fill_pos[0]])
                        fill_pos[0] += 1
                for h2 in range(2):
                    nc.tensor.matmul(
                        opr[h2][0:HS + 1, sub0:512],
                        v_all[:, t, 2 * pair + h2, :],
                        pt[:, h2, sub0:512],
                        start=(t == 0),
                        stop=(t == last),
                    )

            for t in range(last + 1):
                sub0 = max(0, 128 * t - TQC * g)
                diag = t >= 4 * g
                st = stP.tile([128, 2, 512], F32, tag="st",
                              name=f"st{pair}_{g}_{t}")
                pt = ptp.tile([128, 2, 512], BF16, tag="pt",
                              name=f"pt{pair}_{g}_{t}")
                for h2 in range(2):
                    pb = 64 * h2
                    nc.tensor.matmul(
                        st[:, h2, sub0:512],
                        qk[2 + pair][pb:pb + 64, 128 * t:128 * (t + 1)],
                        qk[pair][pb:pb + 64, TQC * g + sub0:TQC * (g + 1)],
                        start=True,
                        stop=not diag,
                    )
                    if diag:
                        nc.tensor.matmul(
                            st[:, h2, sub0:sub0 + 128],
                            imask[:, 0:128],
                            imask[:, 128:256],
                            start=False,
                            stop=True,
                        )
                nc.scalar.activation(
                    out=pt[:, :, sub0:512], in_=st[:, :, sub0:512],
                    func=AF.Exp,
                )
                w = 512 - sub0
                exp_ns = 2 * w * 0.833 + 185
                pe_ns = 4 * w * 0.42 + (128 * 2 * 0.42 if diag else 0)
                ledger[0] += exp_ns - pe_ns
                if t == 0:
                    ledger[0] += 2500  # cover the previous chunk's drain
                pump()
                if pend is not None:
                    emit_av(*pend)
                pend = (t, sub0, pt)
            emit_av(*pend)

            # normalize: 1/l broadcast over the 64 head rows
            for h2 in range(2):
                lr = lrp.tile([1, 512], F32, tag="lr",
                              name=f"lr{pair}_{g}_{h2}")
                nc.vector.reciprocal(out=lr[:], in_=opr[h2][HS:HS + 1, :])
                lb = lbp.tile([64, 512], F32, tag="lb",
                              name=f"lb{pair}_{g}_{h2}")
                nc.gpsimd.partition_broadcast(lb[:], lr[:], channels=64)
                nc.vector.tensor_mul(
                    out=yt[pair][64 * h2:64 * (h2 + 1),
                                 TQC * g:TQC * (g + 1)],
                    in0=opr[h2][0:HS, :],
                    in1=lb[:],
                )

        for g in range(NG):
            attn_chunk(0, g)
        flush()  # remaining v / m(i<2) fillers
        base = len(fillers)
        fillers.extend(deferred)
        for g in range(NG):
            if g in m_deadline:  # qk[1]/qk[3] chunk i=g must exist now
                while fill_pos[0] <= base + m_deadline[g]:
                    emit_filler(fillers[fill_pos[0]])
                    fill_pos[0] += 1
            if g == NG - 1:
                # hold 2 proj tiles back: they don't depend on the final
                # drain, so they cover its latency in the tail
                reserve[0] = 2
            attn_chunk(1, g)
            for tt in range(4 * g, 4 * g + 4):
                fillers.append(("p", tt, 2048 * 0.42))
        tail_mode[0] = 1
        flush()
        poP.release()
        oprP.release()
        stP.release()


_PROG = None


def _get_program():
    global _PROG
    if _PROG is None:
        _PROG = build_program()
    return _PROG


def make_in_maps(x, w_attn, b_attn, w_proj, b_proj):
    x = np.asarray(x, dtype=np.float32)
    w_attn = np.asarray(w_attn, dtype=np.float32)
    b_attn = np.asarray(b_attn, dtype=np.float32)
    w_proj = np.asarray(w_proj, dtype=np.float32)
    s = 1.0 / np.sqrt(HS)
    wq, wk, wv = w_attn[:, 0:C], w_attn[:, C:2 * C], w_attn[:, 2 * C:3 * C]
    bq, bk, bv = b_attn[0:C], b_attn[C:2 * C], b_attn[2 * C:3 * C]
    tri = np.triu(np.ones((128, 128), dtype=np.float32))  # keep tk <= tq
    imask = np.concatenate(
        [np.eye(128, dtype=np.float32), (tri - 1.0) * 30000.0], axis=1)
    in_maps = []
    for core in range(NCORES):
        b, g = divmod(core, 4)
        cs = slice(256 * g, 256 * (g + 1))
        bqk_ = np.stack([bq[cs][0:128] * s, bq[cs][128:256] * s,
                         bk[cs][0:128], bk[cs][128:256]], axis=1)
        in_maps.append({
            "xT": np.ascontiguousarray(x[b].T).astype(NPBF16),
            "wqk": np.ascontiguousarray(
                np.concatenate([wq[:, cs] * s, wk[:, cs]],
                               axis=1)).astype(NPBF16),
            "wv": np.ascontiguousarray(wv[:, cs]).astype(NPBF16),
            "wp": np.ascontiguousarray(w_proj[cs, :]).astype(NPBF16),
            "bqk": np.ascontiguousarray(bqk_),
            "bvb": np.ascontiguousarray(np.concatenate([
                np.broadcast_to(bv[cs][None, :], (128, 256)),
                np.ones((128, 64), dtype=np.float32)], axis=1)),
            "imask": imask.astype(NPBF16),
        })
    return in_maps


def gather_output(results, b_proj):
    b_proj = np.asarray(b_proj, dtype=np.float32)
    out = np.empty((B, T, C), dtype=np.float32)
    for b in range(B):
        acc = results[4 * b]["out"].astype(np.float32)
        for g in range(1, 4):
            acc = acc + results[4 * b + g]["out"].astype(np.float32)
        out[b] = acc + b_proj[None, :]
    return out


def kernel(x, w_attn, b_attn, w_proj, b_proj):
    nc = _get_program()
    in_maps = make_in_maps(x, w_attn, b_attn, w_proj, b_proj)
    res = run_bass_kernel_spmd(nc, in_maps, core_ids=list(range(NCORES)))
    return gather_output(res.results, b_proj)
